# revision 13
# baseline (speedup 1.0000x reference)
"""GraphNorm Trainium2 kernel (channel-major fp16, fold-tree sums,
depth-1 software-pipelined stats, split DMA rings).

out = weight * (x - mean[batch]*ms) / sqrt(var[batch]+eps) + bias,
per-graph mean/var over nodes; var = E[x^2] - (2*ms - ms^2) * mean^2.

Strategy (8 cores, SPMD, one shared program):
  - Host casts x to fp16, lays it out CHANNEL-MAJOR per core
    [C=128 partitions, padded nodes]; each graph ("slot") is a
    contiguous span padded with zeros to a multiple of PAD. Slot
    lengths are uniform across cores (max over cores after a snake
    deal of size-sorted graphs) so one program serves all 8 cores.
  - Per chunk: sum(x) via fold tree of tensor_tensor adds (2x fp16
    DVE) + grouped tensor_reduce; sum(x^2) via ACT Square+accum_out
    per slot into a dead scratch (only the accumulator is used).
  - Stats are software-pipelined one chunk deep with hand-interleaved
    emission so each engine's static queue holds only ready work:
      DVE:  fold_g0(c) | front(c-1): mom,t2 | folds rest(c) |
            recip,W,B(c-1) | applies(c-1)
      ACT:  squares(c)[0:2] | sqrt(c-1) | squares(c)[2:]
    The sqrt lands a couple of square-slots into window c, by which
    time front(c-1) is long done, and the DVE tail reaches the
    reciprocal after the folds, by which time the sqrt has fired:
    neither engine ever blocks on the other.
  - Apply is one fused DVE tensor_scalar (x*W + B) per slot (4x fp16).
  - Loads ride the sync HWDGE ring (also acting as the load throttle
    via D-buffer recycling); stores are issued from GPSIMD (SWDGE)
    right after the covering applies, so a pending store never parks
    a load and the two streams interleave in the SDMA round-robin.
"""

import sys

sys.path.insert(0, "/opt/trn_rl_repo")

import numpy as np

import concourse.bass as bass
import concourse.bacc as bacc
import concourse.tile as tile
from concourse import mybir
from concourse.bass_utils import run_bass_kernel_spmd

f32 = mybir.dt.float32
f16 = mybir.dt.float16

N, C, B = 500000, 128, 512
EPS = 1e-5
NCORES = 8
CHUNK_MAX = 9216
FIRST_CAPS = [2048, 4096]  # small leading chunks -> fast pipeline fill
LAST_CAPS = [4096, 2048, 2048]         # small trailing chunk -> fast pipeline drain
FOLD_MIN = 24   # stop folding at lengths <= this (or odd)
PAD = 64        # slot padding granularity
STORE_GRAN = 4608  # min columns per store DMA
NDBUF = 4       # D(c) lives for windows c..c+1, one more loading
SQRT_POS = 5    # sqrt(c-1) sits after this many squares of chunk c

_prog_cache = {}


def _plan(batch_np):
    cnt = np.bincount(batch_np, minlength=B).astype(np.int64)
    starts = np.zeros(B + 1, np.int64)
    np.cumsum(cnt, out=starts[1:])
    nz = [g for g in range(B) if cnt[g] > 0]
    order = sorted(nz, key=lambda g: (-int(cnt[g]), g))
    percore = [[] for _ in range(NCORES)]
    for i, g in enumerate(order):
        r, k = divmod(i, NCORES)
        if r % 2:
            k = NCORES - 1 - k
        percore[k].append(g)
    nslot = max(len(p) for p in percore)
    slot_len = []
    for j in range(nslot):
        m = 0
        for p in percore:
            if j < len(p):
                m = max(m, -(-int(cnt[p[j]]) // PAD) * PAD)
        assert m <= CHUNK_MAX, f"graph too large for chunk: {m}"
        slot_len.append(m)
    # slot_len is non-increasing by construction
    slot_off = []
    off = 0
    for L in slot_len:
        slot_off.append(off)
        off += L
    T = off
    chunks = []  # (first_slot, nslots, chunk_off, chunk_len)
    cur0, cur_len = 0, 0
    for j in range(nslot):
        # keep leading chunks small: they gate pipeline fill
        cap = (FIRST_CAPS[len(chunks)]
               if len(chunks) < len(FIRST_CAPS) else CHUNK_MAX)
        if cur_len and cur_len + slot_len[j] > cap:
            chunks.append((cur0, j - cur0, slot_off[cur0], cur_len))
            cur0, cur_len = j, 0
        cur_len += slot_len[j]
    if cur_len:
        chunks.append((cur0, nslot - cur0, slot_off[cur0], cur_len))
    # split small tails off the last chunk so the pipeline drains fast
    for cap in LAST_CAPS:
        (s0, ns, coff, clen) = chunks[-1]
        if ns > 2 and clen > 2 * cap:
            cut, cut_len = ns, 0
            while cut > 1 and cut_len + slot_len[s0 + cut - 1] <= cap:
                cut -= 1
                cut_len += slot_len[s0 + cut]
            if 0 < cut < ns:
                chunks[-1] = (s0, cut, coff, clen - cut_len)
                chunks.append((s0 + cut, ns - cut,
                               slot_off[s0 + cut], cut_len))
    return cnt, starts, percore, slot_len, slot_off, chunks, T


def _build(slot_len, slot_off, chunks, T):
    nslot = len(slot_len)
    A = mybir.AluOpType
    nc = bacc.Bacc()
    xcm = nc.dram_tensor("xcm", [128, T], f16, kind="ExternalInput")
    invr = nc.dram_tensor("invr", [128, 2 * nslot], f32, kind="ExternalInput")
    pb = nc.dram_tensor("pb", [128, 4], f32, kind="ExternalInput")
    outp = nc.dram_tensor("outp", [128, T], f16, kind="ExternalOutput")

    nchunk = len(chunks)

    with tile.TileContext(nc) as tc:
        with tc.tile_pool(name="const", bufs=1) as constp, \
             tc.tile_pool(name="dpool", bufs=NDBUF) as dpool, \
             tc.tile_pool(name="opool", bufs=3) as opool, \
             tc.tile_pool(name="scrp", bufs=1) as scrp, \
             tc.tile_pool(name="statp", bufs=4) as statp:

            invt = constp.tile([128, 2 * nslot], f32)
            nc.sync.dma_start(out=invt, in_=invr.ap()[:, :])
            pbt = constp.tile([128, 4], f32)
            nc.sync.dma_start(out=pbt, in_=pb.ap()[:, :])
            epst = constp.tile([128, 1], f32)
            nc.vector.memset(epst, EPS)
            w_col = pbt[:, 0:1]
            b_col = pbt[:, 1:2]
            negs_col = pbt[:, 2:3]
            coef_col = pbt[:, 3:4]

            SCR = scrp.tile([128, CHUNK_MAX], f16)   # fold scratch
            SQ = scrp.tile([128, CHUNK_MAX], f16)    # dead x^2 scratch

            Dt = [None] * nchunk
            St = [None] * nchunk  # (sums, st, wb) per in-flight chunk

            def load(c):
                (s0, ns, coff, clen) = chunks[c]
                D = dpool.tile([128, CHUNK_MAX], f16, tag="D")
                nc.sync.dma_start(out=D[:, 0:clen],
                                  in_=xcm.ap()[:, coff:coff + clen])
                Dt[c] = D

            def fold_groups(c):
                """Yield per-group fold work for chunk c as closures."""
                (s0, ns, coff, clen) = chunks[c]
                D = Dt[c]
                sums = St[c][0]
                sumx = sums[:, 0:ns]
                i = 0
                while i < ns:
                    L = slot_len[s0 + i]
                    j = i
                    while j < ns and slot_len[s0 + j] == L:
                        j += 1
                    yield (i, j, L)
                    i = j

            def emit_fold(c, grp, scr_off, on_gps=False):
                (s0, ns, coff, clen) = chunks[c]
                D = Dt[c]
                sumx = St[c][0][:, 0:ns]
                (i, j, L) = grp
                gs = j - i
                a = slot_off[s0 + i] - coff
                src = D[:, a:a + gs * L].rearrange("p (s l) -> p s l", l=L)
                Lc = L
                while Lc > FOLD_MIN and Lc % 2 == 0:
                    h = Lc // 2
                    dst = SCR[:, scr_off:scr_off + gs * h].rearrange(
                        "p (s l) -> p s l", l=h)
                    nc.vector.tensor_tensor(
                        out=dst, in0=src[:, :, 0:h], in1=src[:, :, h:Lc],
                        op=A.add)
                    src = dst
                    scr_off += gs * h
                    Lc = h
                nc.vector.tensor_reduce(
                    out=sumx[:, i:j], in_=src,
                    axis=mybir.AxisListType.X, op=A.add)
                return scr_off

            def emit_square(c, i):
                (s0, ns, coff, clen) = chunks[c]
                D = Dt[c]
                sumx2 = St[c][0][:, ns:2 * ns]
                a = slot_off[s0 + i] - coff
                e = a + slot_len[s0 + i]
                nc.scalar.activation(
                    out=SQ[:, a:e], in_=D[:, a:e],
                    func=mybir.ActivationFunctionType.Square,
                    accum_out=sumx2[:, i:i + 1])

            def alloc_stats(c):
                (s0, ns, coff, clen) = chunks[c]
                sums = statp.tile([128, 2 * ns], f32, tag="sums")
                st = statp.tile([128, 4 * ns], f32, tag="st")
                wb = statp.tile([128, 2 * ns], f32, tag="wb")
                St[c] = (sums, st, wb)

            def emit_front(c):
                """DVE: mean/ex2 -> t2 = var."""
                (s0, ns, coff, clen) = chunks[c]
                (sums, st, wb) = St[c]
                mean = st[:, 0:ns]
                ex2 = st[:, ns:2 * ns]
                t2 = st[:, 2 * ns:3 * ns]
                mom = st[:, 0:2 * ns]
                inv2 = invt.rearrange("p (h n) -> p h n",
                                      n=nslot)[:, :, s0:s0 + ns]
                nc.vector.tensor_tensor(
                    out=mom.rearrange("p (h n) -> p h n", n=ns),
                    in0=sums.rearrange("p (h n) -> p h n", n=ns),
                    in1=inv2, op=A.mult)
                nc.vector.scalar_tensor_tensor(out=t2, in0=mean,
                                               scalar=coef_col, in1=mean,
                                               op0=A.mult, op1=A.mult)
                nc.vector.tensor_tensor(out=t2, in0=ex2, in1=t2,
                                        op=A.subtract)

            def emit_sqrt(c):
                (s0, ns, coff, clen) = chunks[c]
                st = St[c][1]
                t2 = st[:, 2 * ns:3 * ns]
                nc.scalar.activation(
                    out=t2, in_=t2,
                    func=mybir.ActivationFunctionType.Sqrt, bias=epst)

            def emit_tail(c):
                """DVE: reciprocal, W, B, applies; GPSIMD stores."""
                (s0, ns, coff, clen) = chunks[c]
                (sums, st, wb) = St[c]
                mean = st[:, 0:ns]
                t2 = st[:, 2 * ns:3 * ns]
                istd = st[:, 3 * ns:4 * ns]
                Wt = wb[:, 0:ns]
                Bt = wb[:, ns:2 * ns]
                D = Dt[c]
                OUT = opool.tile([128, CHUNK_MAX], f16, tag="OUT")

                nc.vector.reciprocal(out=istd, in_=t2)
                nc.vector.tensor_scalar(out=Wt, in0=istd, scalar1=w_col,
                                        scalar2=None, op0=A.mult)
                nc.vector.tensor_tensor(out=t2, in0=mean, in1=Wt, op=A.mult)
                nc.vector.tensor_scalar(out=Bt, in0=t2, scalar1=negs_col,
                                        scalar2=b_col, op0=A.mult, op1=A.add)

                st_done = 0
                for i in range(ns):
                    a = slot_off[s0 + i] - coff
                    e = a + slot_len[s0 + i]
                    nc.vector.tensor_scalar(
                        out=OUT[:, a:e], in0=D[:, a:e],
                        scalar1=Wt[:, i:i + 1], scalar2=Bt[:, i:i + 1],
                        op0=A.mult, op1=A.add)
                    if e - st_done >= STORE_GRAN or i == ns - 1:
                        nc.sync.dma_start(
                            out=outp.ap()[:, coff + st_done:coff + e],
                            in_=OUT[:, st_done:e])
                        st_done = e

            for c in range(min(2, nchunk)):
                load(c)

            for c in range(nchunk + 1):
                cur = c if c < nchunk else None
                prev = c - 1 if c >= 1 else None

                if cur is not None:
                    alloc_stats(cur)
                    groups = list(fold_groups(cur))
                    scr_off = 0
                    # first fold group, then the stats front of prev
                    scr_off = emit_fold(cur, groups[0], scr_off)
                if prev is not None:
                    emit_front(prev)
                if cur is not None:
                    for gi, grp in enumerate(groups[1:], start=1):
                        scr_off = emit_fold(cur, grp, scr_off,
                                            on_gps=(gi == len(groups) - 1
                                                    and len(groups) > 1))
                    ns_cur = chunks[cur][1]
                    for i in range(min(SQRT_POS, ns_cur)):
                        emit_square(cur, i)
                if prev is not None:
                    emit_sqrt(prev)
                if cur is not None:
                    for i in range(min(SQRT_POS, ns_cur), ns_cur):
                        emit_square(cur, i)
                if prev is not None:
                    emit_tail(prev)
                if cur is not None and cur + 2 < nchunk:
                    load(cur + 2)

    nc.finalize()
    return nc


def kernel(x, batch, weight, bias, mean_scale, batch_size):
    x = np.asarray(x, dtype=np.float32)
    batch_np = np.asarray(batch).astype(np.int64)
    w = np.asarray(weight, dtype=np.float32)
    b = np.asarray(bias, dtype=np.float32)
    s = np.asarray(mean_scale, dtype=np.float32)
    assert x.shape == (N, C) and int(batch_size) == B

    cnt, starts, percore, slot_len, slot_off, chunks, T = _plan(batch_np)
    nslot = len(slot_len)

    key = tuple(slot_len)
    if key not in _prog_cache:
        _prog_cache[key] = _build(slot_len, slot_off, chunks, T)
    nc = _prog_cache[key]

    x16 = x.astype(np.float16)
    pbm = np.ascontiguousarray(
        np.stack([w, b, -s, 2.0 * s - s * s], axis=1), dtype=np.float32)

    in_maps = []
    for k in range(NCORES):
        xb = np.zeros((T, C), np.float16)
        invm = np.zeros((2 * nslot,), np.float32)
        for j, g in enumerate(percore[k]):
            a = int(starts[g])
            n = int(cnt[g])
            o = slot_off[j]
            xb[o:o + n] = x16[a:a + n]
            invm[j] = 1.0 / n
            invm[nslot + j] = 1.0 / n
        xcm_np = np.ascontiguousarray(xb.T)
        inv128 = np.ascontiguousarray(
            np.broadcast_to(invm[None, :], (128, 2 * nslot)), dtype=np.float32)
        in_maps.append({"xcm": xcm_np, "invr": inv128, "pb": pbm})

    import os
    kw = {}
    if os.environ.get("GN_TRACE", "0") == "1":
        kw = {"trace": True,
              "tmpdir": os.environ.get("GN_TRACE_DIR") or None}
    res = run_bass_kernel_spmd(nc, in_maps, core_ids=list(range(NCORES)), **kw)
    global last_results
    last_results = res

    out = np.empty((N, C), np.float32)
    for k in range(NCORES):
        op = np.asarray(res.results[k]["outp"])  # [128, T] f16
        opT = np.ascontiguousarray(op.T)
        for j, g in enumerate(percore[k]):
            a = int(starts[g])
            n = int(cnt[g])
            o = slot_off[j]
            out[a:a + n] = opT[o:o + n]
    return out
